# revision 12
# baseline (speedup 1.0000x reference)
"""PVT-style spatial-reduction attention on 8 TRN2 NeuronCores (Bass/Tile).

Strategy: data-parallel over batch (16 images -> 2 per core). Each core runs an
identical single-core program on its shard; no collectives.

Host-side prep (inside kernel(), part of sharding/layout):
  - x transposed to channel-major xT [2, 256, 4096] bf16, im2col-permuted.
  - attention scale hd^-0.5 folded into Wq; LayerNorm gamma/beta folded into
    Wkv algebraically (exact); conv weights pre-transposed per tap (bf16).
  - Wp padded to 4 slices [97, 256]: rows 0-31 / 64-95 = head-pair rows,
    rows 32-63 = 0 (junk kill), row 96 = bp/4 (bias rides the contraction).

Device pipeline per batch (all matmuls bf16):
  qT = Wq^T @ xT                        (feature-major q)
  xr = sum over 16 conv taps of gathered-xT^T @ w_tap (strided-gather lhsT)
  LN over free dim (Pool engine for SBUF-only ops), PE-transpose of x_norm
  kT = Wk^T @ xnT ; v_aug = [xn @ Wv | ones] (ones memset into slot cols)
  per 512-query block, per 4-head group:
    S^T = kT_h^T @ qT_h  (keys on partitions) -> exp on ScalarE -> pt bf16
    T   = v_aug^T @ P^T: rows {O^T(32) | 32-replicated sums} per head,
          2 heads per 128 partitions, 2 pairs side-by-side in free dim
    R   = reciprocal_approx_fast(T[32:128])  (partition-shifted read)
    scr[0:96] = T[0:96] * R  (rows 0-31 / 64-95 = normalized O^T, 32-63 junk)
  out = scr^T @ Wp_pad (K=97 incl const-1.0 bias row)  -> ACT copy -> DMA out
"""

import sys
from contextlib import ExitStack

if "/opt/trn_rl_repo" not in sys.path:
    sys.path.insert(0, "/opt/trn_rl_repo")

import numpy as np
import ml_dtypes

import concourse.bass as bass
import concourse.bacc as bacc
import concourse.tile as tile
from concourse import mybir
from concourse.bass_utils import run_bass_kernel_spmd

N_CORES = 8
B, N, C = 16, 4096, 256
B_LOC = B // N_CORES
H8, HD, M = 8, 32, 256
NBLK, BLK = 8, 512
F32 = mybir.dt.float32
BF16 = mybir.dt.bfloat16
I32 = mybir.dt.int32
AF = mybir.ActivationFunctionType
OP = mybir.AluOpType
AX = mybir.AxisListType

KERNEL_STATS = {}


def _kernel_body(ctx, tc):
    nc = tc.nc

    def din(name, shape, dtype=F32):
        return nc.dram_tensor(name, shape, dtype, kind="ExternalInput").ap()

    xT_d = din("xT", [B_LOC, C, N], BF16)
    wq_d = din("wq", [2, 128, C], BF16)
    wk_d = din("wk", [2, 128, C], BF16)
    wv_d = din("wv", [2, 128, C], BF16)
    srw_d = din("srw", [16, 128, 2 * C], BF16)
    srb_d = din("srb", [128, C])
    bk_d = din("bk", [2, 128, 1])
    bv_d = din("bv", [128, C])
    wpa_d = din("wpa", [4, 97, C], BF16)
    eye_d = din("eye", [128, 128], BF16)
    out = nc.dram_tensor("out", [B_LOC, N, C], F32, kind="ExternalOutput").ap()

    consts = ctx.enter_context(tc.tile_pool(name="consts", bufs=1))
    sb_xT = ctx.enter_context(tc.tile_pool(name="sb_xT", bufs=2))
    sb_qT = ctx.enter_context(tc.tile_pool(name="sb_qT", bufs=2))
    sb_pt = ctx.enter_context(tc.tile_pool(name="sb_pt", bufs=6))
    sb_kv = ctx.enter_context(tc.tile_pool(name="sb_kv", bufs=2))
    sb_va = ctx.enter_context(tc.tile_pool(name="sb_va", bufs=2))
    sb_ln = ctx.enter_context(tc.tile_pool(name="sb_ln", bufs=2))
    sb_R = ctx.enter_context(tc.tile_pool(name="sb_R", bufs=2))
    sb_scr = ctx.enter_context(tc.tile_pool(name="sb_scr", bufs=1))
    sb_st = ctx.enter_context(tc.tile_pool(name="sb_st", bufs=3))
    ps_w = ctx.enter_context(tc.tile_pool(name="ps_w", bufs=2, space="PSUM"))
    ps_T = ctx.enter_context(tc.tile_pool(name="ps_T", bufs=2, space="PSUM"))

    cst = {}

    def cload(name, src, shape, dtype=F32):
        t = consts.tile(shape, dtype, tag=name, name=name)
        nc.sync.dma_start(t[:], src)
        return t

    def emit_consts():
        cst["wq"] = [cload(f"wq{k}", wq_d[k], [128, C], BF16) for k in range(2)]
        cst["wk"] = [cload(f"wk{k}", wk_d[k], [128, C], BF16) for k in range(2)]
        cst["wv"] = [cload(f"wv{k}", wv_d[k], [128, C], BF16) for k in range(2)]
        cst["srw"] = [cload(f"srw{t}", srw_d[t], [128, 2 * C], BF16)
                      for t in range(16)]
        cst["srb"] = cload("srb", srb_d[:, :], [128, C])
        cst["bk"] = [cload(f"bk{k}", bk_d[k], [128, 1]) for k in range(2)]
        cst["bv"] = cload("bv", bv_d[:, :], [128, C])
        cst["wpa"] = [cload(f"wpa{i}", wpa_d[i], [97, C], BF16)
                      for i in range(4)]
        cst["eye"] = cload("eye", eye_d[:, :], [128, 128], BF16)
        magic_t = consts.tile([128, 1], I32, tag="magic", name="magic")
        nc.gpsimd.memset(magic_t[:], 0x5F3759DF)
        cst["magic"] = magic_t
        c15_t = consts.tile([128, 1], F32, tag="c15", name="c15")
        nc.gpsimd.memset(c15_t[:], 1.5)
        cst["c15"] = c15_t

    # Per-batch state carried across chunks
    S = [dict() for _ in range(B_LOC)]

    def chunk_load_x(b):
        s = S[b]
        s["xT"] = [sb_xT.tile([128, N], BF16, tag="xT", name=f"xt{b}{k}")
                   for k in range(2)]
        for k in range(2):
            for q4 in range(4):
                nc.sync.dma_start(s["xT"][k][:, 1024 * q4:1024 * (q4 + 1)],
                                  xT_d[b, 128 * k:128 * (k + 1),
                                       1024 * q4:1024 * (q4 + 1)])

    def _conv_mo(b, mo):
        s = S[b]
        psc = ps_w.tile([128, C], F32, tag="w", name=f"psc{b}{mo}")
        for tap in range(16):
            for ki in range(2):
                nc.tensor.matmul(
                    psc[:],
                    s["xT"][ki][:, 256 * tap + 128 * mo:
                                256 * tap + 128 * (mo + 1)],
                    cst["srw"][tap][:, C * ki:C * (ki + 1)],
                    start=(tap == 0 and ki == 0),
                    stop=(tap == 15 and ki == 1),
                )
        return psc

    def chunk_conv0(b):
        S[b]["psc0"] = _conv_mo(b, 0)

    def _ln(b, mo, psc):
        gp = nc.vector
        xb = sb_ln.tile([128, C], F32, tag="xb", name=f"xb{b}{mo}")
        nc.vector.tensor_add(xb[:], psc[:], cst["srb"][:])
        ssum = sb_ln.tile([128, 1], F32, tag="ssum", name=f"ssum{b}{mo}")
        nc.vector.tensor_reduce(ssum[:], xb[:], axis=AX.X, op=OP.add)
        mu = sb_ln.tile([128, 1], F32, tag="mu", name=f"mu{b}{mo}")
        gp.tensor_scalar_mul(mu[:], ssum[:], 1.0 / C)
        xc = sb_ln.tile([128, C], F32, tag="xc", name=f"xc{b}{mo}")
        gp.tensor_scalar_sub(xc[:], xb[:], mu[:, 0:1])
        sq = sb_ln.tile([128, C], F32, tag="sq", name=f"sq{b}{mo}")
        vraw = sb_ln.tile([128, 1], F32, tag="vraw", name=f"vraw{b}{mo}")
        gp.scalar_tensor_tensor(
            sq[:], xc[:], 0.0, xc[:], op0=OP.add, op1=OP.mult,
            accum_out=vraw[:, 0:1])
        veps = sb_ln.tile([128, 1], F32, tag="veps", name=f"veps{b}{mo}")
        gp.tensor_scalar(veps[:], vraw[:], 1.0 / C, 1e-5,
                         op0=OP.mult, op1=OP.add)
        vh = sb_ln.tile([128, 1], F32, tag="vh", name=f"vh{b}{mo}")
        gp.tensor_scalar_mul(vh[:], veps[:], -0.5)
        sh = sb_ln.tile([128, 1], I32, tag="sh", name=f"sh{b}{mo}")
        gp.tensor_scalar(sh[:], veps[:].bitcast(I32), 1, None,
                         op0=OP.logical_shift_right)
        y = sb_ln.tile([128, 1], F32, tag="y", name=f"y{b}{mo}")
        gp.scalar_tensor_tensor(
            y[:].bitcast(I32), cst["magic"][:], 0, sh[:],
            op0=OP.bypass, op1=OP.subtract)
        for it in range(3):
            yy = sb_ln.tile([128, 1], F32, tag=f"yy{it}", name=f"yy{b}{mo}{it}")
            gp.tensor_mul(yy[:], y[:], y[:])
            t2 = sb_ln.tile([128, 1], F32, tag=f"t2{it}", name=f"t2{b}{mo}{it}")
            gp.scalar_tensor_tensor(
                t2[:], yy[:], vh[:, 0:1], cst["c15"][:],
                op0=OP.mult, op1=OP.add)
            y2 = sb_ln.tile([128, 1], F32, tag=f"y2{it}", name=f"yn{b}{mo}{it}")
            gp.tensor_mul(y2[:], y[:], t2[:])
            y = y2
        xn = sb_ln.tile([128, C], BF16, tag="xn", name=f"xn{b}{mo}")
        gp.tensor_scalar_mul(xn[:], xc[:], y[:, 0:1])
        return xn

    def chunk_conv1_ln0(b):
        s = S[b]
        s["psc1"] = _conv_mo(b, 1)
        s["xn0"] = _ln(b, 0, s["psc0"])

    def chunk_kv(b):
        s = S[b]
        s["xn1"] = _ln(b, 1, s["psc1"])
        xn_sb = [s["xn0"], s["xn1"]]
        xnT_sb = []
        for i in range(2):
            xnT = sb_kv.tile([128, M], BF16, tag=f"xnT{i}", name=f"xnT{b}{i}")
            ps_t2 = ps_w.tile([128, M], BF16, tag="w", name=f"pst{b}{i}")
            for j in range(2):
                nc.tensor.transpose(ps_t2[:, 128 * j:128 * (j + 1)],
                                    xn_sb[j][:, 128 * i:128 * (i + 1)],
                                    cst["eye"][:])
            nc.vector.tensor_copy(xnT[:], ps_t2[:])
            xnT_sb.append(xnT)
        # kT per sg: [128 feat, 256 kv-tokens]
        ps_k2 = ps_w.tile([128, 2 * M], F32, tag="w", name=f"psk{b}")
        for mo in range(2):
            for ki in range(2):
                nc.tensor.matmul(
                    ps_k2[:, M * mo:M * (mo + 1)],
                    cst["wk"][ki][:, 128 * mo:128 * (mo + 1)],
                    xnT_sb[ki][:], start=(ki == 0), stop=(ki == 1))
        kT_sb = []
        for mo in range(2):
            kT = sb_kv.tile([128, M], BF16, tag=f"kT{mo}", name=f"kT{b}{mo}")
            nc.vector.tensor_scalar_add(kT[:], ps_k2[:, M * mo:M * (mo + 1)],
                                        cst["bk"][mo][:, 0:1])
            kT_sb.append(kT)
        # v_aug [128 kv-tokens, 8 heads x (32 v | 32 ones)] per ko
        ps_v = ps_w.tile([128, 2 * C], F32, tag="w", name=f"psv{b}")
        for mo in range(2):
            for ki in range(2):
                nc.tensor.matmul(
                    ps_v[:, C * mo:C * (mo + 1)],
                    xnT_sb[ki][:, 128 * mo:128 * (mo + 1)],
                    cst["wv"][ki][:], start=(ki == 0), stop=(ki == 1))
        va_sb = []
        for mo in range(2):
            va = sb_va.tile([128, 512], BF16, tag=f"va{mo}", name=f"va{b}{mo}")
            vslots = va.rearrange("p (h two o) -> p h two o", h=8, two=2)
            nc.gpsimd.memset(vslots[:, :, 1, :], 1.0)
            nc.vector.tensor_add(
                vslots[:, :, 0, :],
                ps_v[:, C * mo:C * (mo + 1)], cst["bv"][:])
            va_sb.append(va)
        s["kT"] = kT_sb
        s["va"] = va_sb

    def _q_blocks(b, blks):
        s = S[b]
        if "qT" not in s:
            s["qT"] = sb_qT.tile([128, 2 * N], BF16, tag="qT", name=f"qT{b}")
        for blk in blks:
            psq = ps_w.tile([128, 2 * BLK], F32, tag="w", name=f"psq{b}{blk}")
            for mo in range(2):
                for ki in range(2):
                    nc.tensor.matmul(
                        psq[:, BLK * mo:BLK * (mo + 1)],
                        cst["wq"][ki][:, 128 * mo:128 * (mo + 1)],
                        s["xT"][ki][:, BLK * blk:BLK * (blk + 1)],
                        start=(ki == 0), stop=(ki == 1))
            qv = s["qT"].rearrange("p (m n) -> p m n", m=2)
            nc.vector.tensor_copy(
                qv[:, :, BLK * blk:BLK * (blk + 1)],
                psq.rearrange("p (m n) -> p m n", m=2))

    A_CHUNKS = [
        chunk_load_x,
        chunk_conv0,
        chunk_conv1_ln0,
        lambda b: _q_blocks(b, range(0, 4)),
        lambda b: _q_blocks(b, range(4, 8)),
        chunk_kv,
    ]

    # 4 persistent scratch tiles (normalized O^T + const-1.0 bias row 96),
    # ring of 2 per sg; WAR/RAW ordering handled by tile dep tracking.
    scr_t = [[sb_scr.tile([97, 2 * BLK], BF16, tag=f"scr{sg}{par}",
                          name=f"scr{sg}{par}")
              for par in range(2)] for sg in range(2)]

    def emit_proj(b, pblk):
        scr01 = [scr_t[sg][pblk % 2] for sg in range(2)]
        for half in range(2):
            P = ps_w.tile([128, 2 * C], F32, tag="w",
                          name=f"pspj{b}{pblk}{half}")
            for w01 in range(2):
                w = 2 * half + w01
                for si in range(4):
                    sg, pr = si >> 1, si & 1
                    nc.tensor.matmul(
                        P[:, C * w01:C * (w01 + 1)],
                        scr01[sg][0:97, 512 * pr + 128 * w:
                                  512 * pr + 128 * (w + 1)],
                        cst["wpa"][si][:],
                        start=(si == 0), stop=(si == 3))
            st = sb_st.tile([128, 2 * C], F32, tag="st",
                            name=f"st{b}{pblk}{half}")
            nc.scalar.copy(st[:], P[:])
            r0 = 512 * pblk + 256 * half
            dst = out[b, r0:r0 + 256, :].rearrange("(p r) c -> r p c", p=2)
            nc.sync.dma_start(dst, st.rearrange("r (p c) -> r p c", p=2))

    def emit_block(b, blk):
        s = S[b]
        kT_sb, qT = s["kT"], s["qT"]
        for sg in range(2):
            pts = []
            for hl in range(4):
                st_t = ps_w.tile([128, 2 * BLK], F32, tag="w",
                                 name=f"psst{b}{blk}{sg}{hl}")
                for ko in range(2):
                    nc.tensor.matmul(
                        st_t[:, BLK * ko:BLK * (ko + 1)],
                        kT_sb[sg][32 * hl:32 * hl + 32,
                                  128 * ko:128 * (ko + 1)],
                        qT[32 * hl:32 * hl + 32,
                           N * sg + BLK * blk:N * sg + BLK * (blk + 1)],
                        start=True, stop=True,
                        tile_position=(32 * hl, 0),
                    )
                pt = sb_pt.tile([128, 2 * BLK], BF16, tag="pt",
                                name=f"pt{b}{blk}{sg}{hl}")
                nc.scalar.activation(pt[:], st_t[:], AF.Exp)
                pts.append(pt)
            T = ps_T.tile([128, 2 * BLK], F32, tag="T", name=f"T{b}{blk}{sg}")
            for hl in range(4):
                pair, slot = hl >> 1, hl & 1
                hh = 4 * sg + hl
                for ko in range(2):
                    nc.tensor.matmul(
                        T[64 * slot:64 * slot + 64,
                          BLK * pair:BLK * (pair + 1)],
                        s["va"][ko][:, 64 * hh:64 * hh + 64],
                        pts[hl][:, BLK * ko:BLK * (ko + 1)],
                        start=(ko == 0), stop=(ko == 1),
                        tile_position=(0, 64 * slot),
                        skip_group_check=True,
                    )
            # Partition-base rule: >32-partition APs must start at 0 (or 64
            # for <=64). Full-tile recip, then two 32-partition muls at
            # legal bases; scratch rows 32-63 are preset to 0.
            R4 = sb_R.tile([128, 2 * BLK], F32, tag="R", name=f"R{b}{blk}{sg}")
            nc.vector.reciprocal_approx_fast(R4[:], T[:])
            scr = scr_t[sg][blk % 2]
            nc.vector.tensor_mul(scr[0:32, :], T[0:32, :], R4[32:64, :])
            nc.vector.tensor_mul(scr[64:96, :], T[64:96, :], R4[96:128, :])
        if blk >= 1:
            emit_proj(b, blk - 1)

    # ---------- emission schedule ----------
    emit_consts()
    chunk_load_x(0)
    # preset scratch tiles: row 96 = const-1.0 bias row; rows 32-63 = 0
    # (never written afterwards -> proj junk contribution is exactly 0)
    for sg in range(2):
        for par in range(2):
            nc.gpsimd.memset(scr_t[sg][par][96:97, :], 1.0)
            nc.gpsimd.memset(scr_t[sg][par][32:64, :], 0.0)
    for f in A_CHUNKS[1:]:
        f(0)
    for b in range(B_LOC):
        for blk in range(NBLK):
            emit_block(b, blk)
            if b + 1 < B_LOC and blk < len(A_CHUNKS):
                A_CHUNKS[blk](b + 1)
        emit_proj(b, NBLK - 1)


def build():
    nc = bacc.Bacc("TRN2", target_bir_lowering=False, debug=False,
                   enable_asserts=True)
    with tile.TileContext(nc) as tc:
        with ExitStack() as ctx:
            _kernel_body(ctx, tc)
    nc.compile()
    return nc


def host_prep(inputs):
    """Shared (non-x) host-side tensors, from the full input dict."""
    Wq = np.asarray(inputs["Wq"], np.float32)
    Wkv = np.asarray(inputs["Wkv"], np.float32)
    sr_w = np.asarray(inputs["sr_w"], np.float32)
    sr_b = np.asarray(inputs["sr_b"], np.float32)
    ln_g = np.asarray(inputs["ln_g"], np.float32)
    ln_b = np.asarray(inputs["ln_b"], np.float32)
    Wp = np.asarray(inputs["Wp"], np.float32)
    bp = np.asarray(inputs["bp"], np.float32)
    bf = ml_dtypes.bfloat16

    wq = (Wq * (HD ** -0.5)).reshape(2, 128, C).astype(bf)
    wk = (ln_g[:, None] * Wkv[:, :C]).reshape(2, 128, C).astype(bf)
    wv = (ln_g[:, None] * Wkv[:, C:]).reshape(2, 128, C).astype(bf)
    bias_kv = (ln_b @ Wkv).astype(np.float32)
    srwT = sr_w.transpose(2, 3, 1, 0).reshape(16, C, C)
    # srw8[tap, p, ki*256 + o] = srwT[tap, ki*128 + p, o]
    srw8 = np.ascontiguousarray(
        srwT.reshape(16, 2, 128, C).transpose(0, 2, 1, 3).reshape(16, 128, 2 * C)
    ).astype(bf)

    wpa = np.zeros((4, 97, C), np.float32)
    for si in range(4):
        sg, pr = si >> 1, si & 1
        h_even, h_odd = 4 * sg + 2 * pr, 4 * sg + 2 * pr + 1
        wpa[si, 0:32] = Wp[32 * h_even:32 * h_even + 32]
        wpa[si, 64:96] = Wp[32 * h_odd:32 * h_odd + 32]
        wpa[si, 96] = bp / 4.0

    shared = {
        "wq": wq,
        "wk": wk,
        "wv": wv,
        "srw": srw8,
        "srb": np.ascontiguousarray(np.broadcast_to(sr_b, (128, C))),
        "bk": np.ascontiguousarray(bias_kv[:C].reshape(2, 128, 1)),
        "bv": np.ascontiguousarray(np.broadcast_to(bias_kv[C:], (128, C))),
        "wpa": wpa.astype(bf),
        "eye": np.eye(128, dtype=np.float32).astype(bf),
    }
    return shared


_NC_CACHE = {}


def get_nc(with_bp=False):
    if "nc" not in _NC_CACHE:
        _NC_CACHE["nc"] = build()
    return _NC_CACHE["nc"]


def _im2col_perm():
    """idx[tap*256 + m] = spatial row index n for the stride-4 4x4 conv."""
    tap = np.arange(16)
    kh, kw = tap // 4, tap % 4
    m = np.arange(256)
    R, Cc = m // 16, m % 16
    idx = (256 * R[None, :] + 4 * Cc[None, :]
           + 64 * kh[:, None] + kw[:, None])
    return idx.reshape(-1)


IM2COL_IDX = _im2col_perm()


def make_in_maps(inputs):
    x = np.asarray(inputs["x"], np.float32)
    shared = host_prep(inputs)
    in_maps = []
    for c in range(N_CORES):
        xc = x[B_LOC * c:B_LOC * (c + 1)]
        xT = np.ascontiguousarray(
            xc.transpose(0, 2, 1)[:, :, IM2COL_IDX]).astype(ml_dtypes.bfloat16)
        m = dict(shared)
        m["xT"] = xT
        in_maps.append(m)
    return in_maps, False


def kernel(**inputs):
    in_maps, _ = make_in_maps(inputs)
    nc = get_nc()
    res = run_bass_kernel_spmd(nc, in_maps, core_ids=list(range(N_CORES)))
    KERNEL_STATS["exec_time_ns"] = res.exec_time_ns
    KERNEL_STATS["mean_exec_time_ns"] = res.mean_exec_time_ns
    KERNEL_STATS["trace"] = res.instructions_and_trace
    out_perm = np.concatenate(
        [res.results[c]["out"] for c in range(N_CORES)], axis=0)
    out = np.empty_like(out_perm)
    out[:, IM2COL_IDX, :] = out_perm
    return out
